# revision 4
# baseline (speedup 1.0000x reference)
"""Trainium2 Bass kernel for nn_MultiHeadAttention_72765335929540.

Reference semantics (B=8, S=2048, D=512, H=8 identical heads, d_k=d_v=64):
    q = query @ Wq + bq;  k = key @ Wk + bk;  v = key @ Wv + bv   (bug: v from key)
    scores = q k^T / 8 (+ causal mask if training);  att = softmax(scores)
    head = att @ v;  out = tile(head, 8) @ Wo + bo = head @ Wo_eff + bo
where Wo_eff = sum_h Wo[64h:64h+64].  `value` is never read.

Distribution: data-parallel, one batch element per NeuronCore (8 cores).

v2 design (vs v1): inputs are host-transposed and host-cast to bf16
(qT/kT [D, S]), so the device does no PE transposes of X and no casting
DMAs.  Loads go through HWDGE (sync engine) in column-half chunks so the
first projection can start ~4us after launch.  Output is stored bf16 and
upcast on host.  Engine budget: PE does projections/scores/heads/out,
ACT does only the exp, DVE does bias evictions + reciprocals + half the
out evictions, Pool does v' evictions + the other half.

Per-core pipeline (bf16 compute, f32 accumulate in PSUM):
  1. qT_sb = Wq^T X_q^T (+bq on eviction);  kvT = [Wk|Wv]^T X_k^T (+bias)
  2. v' = [v | 1] via PE re-transpose of vT (ones column -> softmax denom)
  3. per key-block J: scoresT[j,i] = kT_J^T qT (PE), pT = exp(scoresT/8)
     (ACT, no max-subtraction -- scores are provably < ~3), causal diag
     mask via an accumulated -1e30 upper-triangle matmul
  4. headT'[d,i] (d<64: sum_j v pT; d=64: denominator l_i) on PE
  5. out_b = (headT'^T @ [Wo_eff; bo]) * (1/l_i) -- norm + bias fused

PSUM budget (8 banks): sc x4 (proj + scoresT pieces), ha x1 (headT'
acc), pl x1 (v' transposes + l column), po x2 (final out psum).
"""
import sys

sys.path.insert(0, "/opt/trn_rl_repo")

import numpy as np
import ml_dtypes

import concourse.bass as bass
import concourse.mybir as mybir
import concourse.tile as tile
from concourse.bass_utils import run_bass_kernel_spmd

BF = mybir.dt.bfloat16
F32 = mybir.dt.float32
S, D, DK = 2048, 512, 64
NB = S // 128          # 16 blocks of 128
H = 8

# ---------------------------------------------------------------------------
# walrus workaround: this build's ISA structs hold few semaphore waits per
# instruction; split the excess onto same-engine NoOps (1 wait each).
_ws_counter = [0]
_CTRL_TYPES = ("InstDrain", "InstNoOp", "InstEventSemaphore", "InstBranch")


def _split_sync_waits(nc, max_waits=1, max_updates=2):
    for f in nc.m.functions:
        for blk in f.blocks:
            insts = blk.instructions
            i = 0
            while i < len(insts):
                inst = insts[i]
                si = inst.sync_info
                if si is None:
                    i += 1
                    continue
                ctrl = type(inst).__name__ in _CTRL_TYPES
                max_w = 1 if ctrl else max_waits
                max_u = 1 if ctrl else max_updates
                waits = list(si.on_wait)
                updates = list(si.on_update)
                if len(waits) <= max_w and len(updates) <= max_u:
                    i += 1
                    continue
                keep_w = waits[-max_w:] if len(waits) > max_w else waits
                extra_w = waits[:-max_w] if len(waits) > max_w else []
                keep_u = updates[:max_u] if len(updates) > max_u else updates
                extra_u = updates[max_u:] if len(updates) > max_u else []
                inst.sync_info = mybir.SyncInfo(on_wait=keep_w, on_update=keep_u)
                pre, post = [], []
                for w in extra_w:
                    _ws_counter[0] += 1
                    nop = mybir.InstNoOp(name=f"WSPLIT-{_ws_counter[0]}", ins=[], outs=[])
                    nop.engine = inst.engine
                    nop.sync_info = mybir.SyncInfo(on_wait=[w], on_update=[])
                    pre.append(nop)
                for u in extra_u:
                    _ws_counter[0] += 1
                    nop = mybir.InstNoOp(name=f"USPLIT-{_ws_counter[0]}", ins=[], outs=[])
                    nop.engine = inst.engine
                    nop.sync_info = mybir.SyncInfo(on_wait=[], on_update=[u])
                    post.append(nop)
                for k, nop in enumerate(pre):
                    insts.insert(i + k, nop)
                for k, nop in enumerate(post):
                    insts.insert(i + len(pre) + 1 + k, nop)
                i += len(pre) + 1 + len(post)


# ---------------------------------------------------------------------------
def _build_nc(masked: bool):
    nc = bass.Bass()
    # host-transposed, host-cast inputs: [D, S] bf16, split in column halves
    qt_d = nc.declare_dram_parameter("qt", [D, S], BF, isOutput=False)
    kt_d = nc.declare_dram_parameter("kt", [D, S], BF, isOutput=False)
    wq_d = nc.declare_dram_parameter("wq", [D, DK], BF, isOutput=False)
    wkv_d = nc.declare_dram_parameter("wkv", [D, 128], BF, isOutput=False)
    bq_d = nc.declare_dram_parameter("bq", [DK, 1], F32, isOutput=False)
    bkv_d = nc.declare_dram_parameter("bkv", [128, 1], F32, isOutput=False)
    frhs_d = nc.declare_dram_parameter("frhs", [DK + 1, D], BF, isOutput=False)
    trineg_d = nc.declare_dram_parameter("trineg", [128, 128], BF, isOutput=False)
    id_d = nc.declare_dram_parameter("ident", [128, 128], BF, isOutput=False)
    out_d = nc.declare_dram_parameter("out", [S, D], BF, isOutput=True)
    warm_d = nc.declare_dram_parameter("warm", [128, 1], F32, isOutput=True)

    Exp = mybir.ActivationFunctionType.Exp

    with tile.TileContext(nc) as tc:
        with (
            tc.tile_pool(name="pers", bufs=1) as pers,
            tc.tile_pool(name="hts", bufs=3) as hts,
            tc.tile_pool(name="osb", bufs=2) as osb,
            tc.tile_pool(name="ps", bufs=2, space="PSUM") as ps,
        ):
            # ---- persistent SBUF ------------------------------------------
            # xT tiles: [128, 4 chunks, 1024 cols] per column-half h
            xq = [pers.tile([128, 4, S // 2], BF, tag=f"xq{h}", name=f"xq{h}")
                  for h in range(2)]
            xk = [pers.tile([128, 4, S // 2], BF, tag=f"xk{h}", name=f"xk{h}")
                  for h in range(2)]
            wq_sb = pers.tile([128, 4 * DK], BF, tag="wq")
            wkv_sb = pers.tile([128, 4 * 128], BF, tag="wkv")
            bq_sb = pers.tile([DK, 1], F32, tag="bq")
            bkv_sb = pers.tile([128, 1], F32, tag="bkv")
            frhs_sb = pers.tile([DK + 1, D], BF, tag="frhs")
            trineg_sb = pers.tile([128, 128], BF, tag="trineg")
            id_sb = pers.tile([128, 128], BF, tag="id")
            qT = pers.tile([DK, S], BF, tag="qT")
            kvT = pers.tile([128, S], BF, tag="kvT")
            vprime = [pers.tile([128, DK + 1], BF, tag=f"vp{j}", name=f"vp{j}")
                      for j in range(NB)]
            Ws = [(S - 128 * J) if masked else S for J in range(NB)]
            pts = [pers.tile([128, Ws[J]], BF, tag=f"pt{J}", name=f"pt_{J}")
                   for J in range(NB)]
            wu = pers.tile([128, 512], BF, tag="wu")
            wu2 = pers.tile([128, 1], F32, tag="wu2")

            # ---- loads: big HWDGE transfers issued from sync --------------
            # order: q half 0, consts, q half 1, k half 0, k half 1
            def load_half(x_sb, src_d, h):
                # DRAM [512, 1024] col-slice -> SBUF [128, 4, 1024]
                nc.sync.dma_start(
                    x_sb[:],
                    src_d[:, h * (S // 2):(h + 1) * (S // 2)].rearrange(
                        "(c p) i -> p c i", p=128))

            load_half(xq[0], qt_d, 0)
            for cc in range(4):
                nc.sync.dma_start(wq_sb[:, cc * DK:(cc + 1) * DK],
                                  wq_d[cc * 128:(cc + 1) * 128, :])
                nc.sync.dma_start(wkv_sb[:, cc * 128:(cc + 1) * 128],
                                  wkv_d[cc * 128:(cc + 1) * 128, :])
            nc.sync.dma_start(bq_sb[:], bq_d[:])
            nc.sync.dma_start(bkv_sb[:], bkv_d[:])
            nc.sync.dma_start(frhs_sb[:], frhs_d[:])
            nc.sync.dma_start(trineg_sb[:], trineg_d[:])
            nc.sync.dma_start(id_sb[:], id_d[:])
            load_half(xq[1], qt_d, 1)
            load_half(xk[0], kt_d, 0)
            load_half(xk[1], kt_d, 1)

            # ---- PE warm-up: junk matmuls while the first DMAs fly --------
            # The HAM clock gate keeps PE at 1.2 GHz until ~3.4us of
            # sustained activity; open it before the real work lands.
            nc.vector.memset(wu[:], 0.0)
            wu_ps = ps.tile([128, 512], F32, tag="sc", name="wu_ps", bufs=4)
            for i in range(14):
                nc.tensor.matmul(wu_ps[:], lhsT=wu[:, 0:128], rhs=wu[:],
                                 start=(i == 0), stop=(i == 13))
            nc.vector.tensor_copy(wu2[:], wu_ps[:, 0:1])

            def keepalive(n, who, cols=256):
                kps = ps.tile([128, 512], F32, tag="sc", name=f"ka_{who}", bufs=4)
                for i in range(n):
                    nc.tensor.matmul(kps[:, 0:cols], lhsT=wu[:, 0:128],
                                     rhs=wu[:, 0:cols],
                                     start=(i == 0), stop=(i == n - 1))

            # ones columns for v' (Pool, early, no deps)
            for jb in range(NB):
                nc.gpsimd.memset(vprime[jb][:, DK:DK + 1], 1.0)

            # ---- projections ----------------------------------------------
            def query_proj(p):
                sl = slice(p * 512, (p + 1) * 512)
                h, off = divmod(p * 512, S // 2)
                pq = ps.tile([DK, 512], F32, tag="sc", name=f"pq_{p}", bufs=4)
                for cc in range(4):
                    nc.tensor.matmul(pq[:],
                                     lhsT=wq_sb[:, cc * DK:(cc + 1) * DK],
                                     rhs=xq[h][:, cc, off:off + 512],
                                     start=(cc == 0), stop=(cc == 3))
                nc.vector.tensor_scalar_add(qT[:, sl], pq[:], bq_sb[:, 0:1])

            def kv_proj(p):
                sl = slice(p * 512, (p + 1) * 512)
                h, off = divmod(p * 512, S // 2)
                pkv = ps.tile([128, 512], F32, tag="sc", name=f"pkv_{p}", bufs=4)
                for cc in range(4):
                    nc.tensor.matmul(pkv[:],
                                     lhsT=wkv_sb[:, cc * 128:(cc + 1) * 128],
                                     rhs=xk[h][:, cc, off:off + 512],
                                     start=(cc == 0), stop=(cc == 3))
                nc.vector.tensor_scalar_add(kvT[:, sl], pkv[:], bkv_sb[:, 0:1])
                # v' for the 4 j-blocks of this piece
                for t in range(4):
                    jb = p * 4 + t
                    pv = ps.tile([128, DK], BF, tag="pl", name=f"pv_{jb}", bufs=1)
                    nc.tensor.transpose(pv[:],
                                        kvT[64:128, jb * 128:(jb + 1) * 128],
                                        id_sb[64:128, 64:128])
                    nc.vector.tensor_copy(vprime[jb][:, 0:DK], pv[:])

            # ---- scores + exp for sweep p ---------------------------------
            def scores(p):
                Jmax = 4 * p + 3 if masked else NB - 1
                for J in range(0, Jmax + 1):
                    pt = pts[J]
                    i_start = max(512 * p, 128 * J) if masked else 512 * p
                    w = 512 * p + 512 - i_start
                    x0 = i_start - (128 * J if masked else 0)
                    psc = ps.tile([128, 512], F32, tag="sc", name=f"sc_{J}_{p}",
                                  bufs=4)
                    diag = masked and J // 4 == p
                    nc.tensor.matmul(psc[:, 0:w],
                                     lhsT=kvT[0:DK, J * 128:(J + 1) * 128],
                                     rhs=qT[:, i_start:i_start + w],
                                     start=True, stop=not diag,
                                     skip_group_check=True)
                    if diag:
                        # accumulate -1e30 upper-triangle into the diag block
                        nc.tensor.matmul(psc[:, 0:128], lhsT=id_sb[:],
                                         rhs=trineg_sb[:], start=False, stop=True,
                                         skip_group_check=True)
                    nc.scalar.activation(pt[:, x0:x0 + w], psc[:, 0:w],
                                         Exp, scale=0.125)

            # ---- head accumulation + finalize for sweep p -----------------
            def heads(p):
                Jmax = 4 * p + 3 if masked else NB - 1
                hacc = ps.tile([DK + 1, 512], F32, tag="ha", name=f"ha_{p}", bufs=1)
                for J in range(0, Jmax + 1):
                    b_lo = max(4 * p, J) if masked else 4 * p
                    wdt = (4 * p + 4 - b_lo) * 128
                    c0 = (b_lo % 4) * 128
                    x = (128 * (b_lo - J) if masked else 512 * p)
                    nc.tensor.matmul(hacc[:, c0:c0 + wdt],
                                     lhsT=vprime[J][:], rhs=pts[J][:, x:x + wdt],
                                     start=(J == 0), stop=(J == Jmax),
                                     skip_group_check=True)
                ht4 = hts.tile([DK + 1, 512], BF, tag="ht", name=f"ht4_{p}")
                nc.vector.tensor_copy(ht4[:], hacc[:])
                # finalize: per 128-row block, normalize + out projection
                ot4 = osb.tile([128, 4, D], BF, tag="ot", name=f"ot4_{p}")
                for b in range(4 * p, 4 * p + 4):
                    c0 = (b % 4) * 128
                    pl = ps.tile([128, 1], BF, tag="pl", name=f"pl_{b}", bufs=1)
                    nc.tensor.transpose(pl[:], ht4[DK:DK + 1, c0:c0 + 128],
                                        id_sb[64:65, 64:65])
                    r = hts.tile([128, 1], F32, tag="r", name=f"r_{b}")
                    nc.vector.reciprocal(r[:], pl[:, 0:1])
                    po = ps.tile([128, 512], F32, tag="po", name=f"po_{b}", bufs=2)
                    nc.tensor.matmul(po[:], lhsT=ht4[:, c0:c0 + 128], rhs=frhs_sb[:],
                                     start=True, stop=True)
                    if b % 2 == 0:
                        nc.vector.tensor_scalar_mul(ot4[:, b % 4, :], po[:], r[:, 0:1])
                    else:
                        nc.scalar.mul(ot4[:, b % 4, :], po[:], r[:, 0:1])
                nc.sync.dma_start(
                    out_d[p * 512:(p + 1) * 512, :].rearrange(
                        "(c p) i -> p c i", p=128),
                    ot4[:])

            # ---- program: fill PE while loads land, lag heads one sweep ---
            query_proj(0)
            query_proj(1)
            keepalive(4, "kq")
            query_proj(2)
            query_proj(3)
            keepalive(4, "kk")
            kv_proj(0)
            scores(0)
            kv_proj(1)
            scores(1)
            heads(0)
            kv_proj(2)
            scores(2)
            heads(1)
            kv_proj(3)
            scores(3)
            heads(2)
            heads(3)
            nc.gpsimd.dma_start(warm_d[:], wu2[:])

    _split_sync_waits(nc)
    return nc


_NC_CACHE = {}


def _get_nc(masked: bool):
    if masked not in _NC_CACHE:
        _NC_CACHE[masked] = _build_nc(masked)
    return _NC_CACHE[masked]


def _prep_consts(Wq, bq, Wk, bk, Wv, bv, Wo, bo):
    Wq = np.asarray(Wq, dtype=np.float64)
    Wk = np.asarray(Wk, dtype=np.float64)
    Wv = np.asarray(Wv, dtype=np.float64)
    Wo = np.asarray(Wo, dtype=np.float64)
    bq_h = np.asarray(bq, dtype=np.float32).reshape(DK, 1)
    bk_h = np.asarray(bk, dtype=np.float32).reshape(DK, 1)
    bv_h = np.asarray(bv, dtype=np.float32).reshape(DK, 1)
    bo_h = np.asarray(bo, dtype=np.float64)
    wo_eff = Wo.reshape(H, DK, D).sum(axis=0)
    frhs_h = np.concatenate([wo_eff, bo_h[None, :]], axis=0).astype(ml_dtypes.bfloat16)
    jj, ii = np.meshgrid(np.arange(128), np.arange(128), indexing="ij")
    trineg_h = np.where(jj <= ii, 0.0, -1e30).astype(ml_dtypes.bfloat16)
    return {
        "wq": Wq.astype(ml_dtypes.bfloat16),
        "wkv": np.concatenate([Wk, Wv], axis=1).astype(ml_dtypes.bfloat16),
        "bq": bq_h,
        "bkv": np.concatenate([bk_h, bv_h], axis=0),
        "frhs": frhs_h,
        "trineg": trineg_h,
        "ident": np.eye(128, dtype=ml_dtypes.bfloat16),
    }


# ---------------------------------------------------------------------------
def kernel(query, key, value, Wq, bq, Wk, bk, Wv, bv, Wo, bo, training):
    query = np.asarray(query, dtype=np.float32)
    key = np.asarray(key, dtype=np.float32)
    masked = bool(np.asarray(training).item())
    B = query.shape[0]

    consts = _prep_consts(Wq, bq, Wk, bk, Wv, bv, Wo, bo)
    in_maps = [
        dict(consts,
             qt=np.ascontiguousarray(query[i].T).astype(ml_dtypes.bfloat16),
             kt=np.ascontiguousarray(key[i].T).astype(ml_dtypes.bfloat16))
        for i in range(B)
    ]

    nc = _get_nc(masked)
    res = run_bass_kernel_spmd(nc, in_maps, core_ids=list(range(B)))
    return np.stack([np.asarray(res.results[i]["out"]).astype(np.float32)
                     for i in range(B)])


# revision 13
# speedup vs baseline: 1.2099x; 1.2099x over previous
"""Trainium2 Bass kernel for nn_MultiHeadAttention_72765335929540.

Reference semantics (B=8, S=2048, D=512, H=8 identical heads, d_k=d_v=64):
    q = query @ Wq + bq;  k = key @ Wk + bk;  v = key @ Wv + bv   (bug: v from key)
    scores = q k^T / 8 (+ causal mask if training);  att = softmax(scores)
    head = att @ v;  out = tile(head, 8) @ Wo + bo = head @ Wo_eff + bo
where Wo_eff = sum_h Wo[64h:64h+64].  `value` is never read.

Distribution: data-parallel, one batch element per NeuronCore (8 cores).

v2 design (vs v1): inputs are host-transposed and host-cast to bf16
(qT/kT [D, S]), so the device does no PE transposes of X and no casting
DMAs.  Loads go through HWDGE (sync engine) in column-half chunks so the
first projection can start ~4us after launch.  Output is stored bf16 and
upcast on host.  Engine budget: PE does projections/scores/heads/out,
ACT does only the exp, DVE does bias evictions + reciprocals + half the
out evictions, Pool does v' evictions + the other half.

Per-core pipeline (bf16 compute, f32 accumulate in PSUM):
  1. qT_sb = Wq^T X_q^T (+bq on eviction);  kvT = [Wk|Wv]^T X_k^T (+bias)
  2. v' = [v | 1] via PE re-transpose of vT (ones column -> softmax denom)
  3. per key-block J: scoresT[j,i] = kT_J^T qT (PE), pT = exp(scoresT/8)
     (ACT, no max-subtraction -- scores are provably < ~3), causal diag
     mask via an accumulated -1e30 upper-triangle matmul
  4. headT'[d,i] (d<64: sum_j v pT; d=64: denominator l_i) on PE
  5. out_b = (headT'^T @ [Wo_eff; bo]) * (1/l_i) -- norm + bias fused

PSUM budget (8 banks): sc x4 (proj + scoresT pieces), ha x1 (headT'
acc), pl x1 (v' transposes + l column), po x2 (final out psum).
"""
import sys

sys.path.insert(0, "/opt/trn_rl_repo")

import numpy as np
import ml_dtypes

import concourse.bass as bass
import concourse.mybir as mybir
import concourse.tile as tile
from concourse.bass_utils import run_bass_kernel_spmd

BF = mybir.dt.bfloat16
F32 = mybir.dt.float32
S, D, DK = 2048, 512, 64
NB = S // 128          # 16 blocks of 128
H = 8

# ---------------------------------------------------------------------------
# walrus workaround: this build's ISA structs hold few semaphore waits per
# instruction; split the excess onto same-engine NoOps (1 wait each).
_ws_counter = [0]
_CTRL_TYPES = ("InstDrain", "InstNoOp", "InstEventSemaphore", "InstBranch")


def _split_sync_waits(nc, max_waits=1, max_updates=2):
    for f in nc.m.functions:
        for blk in f.blocks:
            insts = blk.instructions
            i = 0
            while i < len(insts):
                inst = insts[i]
                si = inst.sync_info
                if si is None:
                    i += 1
                    continue
                ctrl = type(inst).__name__ in _CTRL_TYPES
                max_w = 1 if ctrl else max_waits
                max_u = 1 if ctrl else max_updates
                waits = list(si.on_wait)
                updates = list(si.on_update)
                if len(waits) <= max_w and len(updates) <= max_u:
                    i += 1
                    continue
                keep_w = waits[-max_w:] if len(waits) > max_w else waits
                extra_w = waits[:-max_w] if len(waits) > max_w else []
                keep_u = updates[:max_u] if len(updates) > max_u else updates
                extra_u = updates[max_u:] if len(updates) > max_u else []
                inst.sync_info = mybir.SyncInfo(on_wait=keep_w, on_update=keep_u)
                pre, post = [], []
                for w in extra_w:
                    _ws_counter[0] += 1
                    nop = mybir.InstNoOp(name=f"WSPLIT-{_ws_counter[0]}", ins=[], outs=[])
                    nop.engine = inst.engine
                    nop.sync_info = mybir.SyncInfo(on_wait=[w], on_update=[])
                    pre.append(nop)
                for u in extra_u:
                    _ws_counter[0] += 1
                    nop = mybir.InstNoOp(name=f"USPLIT-{_ws_counter[0]}", ins=[], outs=[])
                    nop.engine = inst.engine
                    nop.sync_info = mybir.SyncInfo(on_wait=[], on_update=[u])
                    post.append(nop)
                for k, nop in enumerate(pre):
                    insts.insert(i + k, nop)
                for k, nop in enumerate(post):
                    insts.insert(i + len(pre) + 1 + k, nop)
                i += len(pre) + 1 + len(post)


# ---------------------------------------------------------------------------
def _build_nc(masked: bool):
    nc = bass.Bass()
    # host-transposed, host-cast inputs: [D, S] bf16, split in column halves
    qt_d = nc.declare_dram_parameter("qt", [D, S], BF, isOutput=False)
    kt_d = nc.declare_dram_parameter("kt", [D, S], BF, isOutput=False)
    # all constants packed into one tensor -> one DMA:
    # [0:256) wq | [256:768) wkv | [768:1280) frhs (padded) | [1280:1408)
    # trineg | [1408:1536) ident | [1536:1540) bq,bkv as bf16-pair bitcast
    cst_d = nc.declare_dram_parameter("cst", [128, 1540], BF, isOutput=False)
    out_d = nc.declare_dram_parameter("out", [S, D], BF, isOutput=True)
    warm_d = nc.declare_dram_parameter("warm", [128, 1], F32, isOutput=True)

    Exp = mybir.ActivationFunctionType.Exp

    with tile.TileContext(nc) as tc:
        with (
            tc.tile_pool(name="pers", bufs=1) as pers,
            tc.tile_pool(name="hts", bufs=3) as hts,
            tc.tile_pool(name="osb", bufs=2) as osb,
            tc.tile_pool(name="ps", bufs=2, space="PSUM") as ps,
        ):
            # ---- persistent SBUF ------------------------------------------
            # xT tiles: [128, 4 chunks, 1024 cols] per column-half h
            xq = [pers.tile([128, 4, S // 2], BF, tag=f"xq{h}", name=f"xq{h}")
                  for h in range(2)]
            xk = [pers.tile([128, 4, S // 2], BF, tag=f"xk{h}", name=f"xk{h}")
                  for h in range(2)]
            cst_sb = pers.tile([128, 1540], BF, tag="cst")
            wq_sb = cst_sb[:, 0:256]
            wkv_sb = cst_sb[:, 256:768]
            frhs_sb = cst_sb[0:DK + 1, 768:1280]
            trineg_sb = cst_sb[:, 1280:1408]
            id_sb = cst_sb[:, 1408:1536]
            bq_sb = cst_sb[0:DK, 1536:1538].bitcast(F32)
            bkv_sb = cst_sb[:, 1538:1540].bitcast(F32)
            qT = pers.tile([DK, S], BF, tag="qT")
            kvT = pers.tile([128, S], BF, tag="kvT")
            vprime = [pers.tile([128, DK + 1], BF, tag=f"vp{j}", name=f"vp{j}")
                      for j in range(NB)]
            Ws = [(S - 128 * J) if masked else S for J in range(NB)]
            pts = [pers.tile([128, Ws[J]], BF, tag=f"pt{J}", name=f"pt_{J}")
                   for J in range(NB)]
            wu = pers.tile([128, 512], BF, tag="wu")
            wu2 = pers.tile([128, 1], F32, tag="wu2")

            # ---- loads: big HWDGE transfers issued from sync --------------
            # order: q half 0, consts, q half 1, k half 0, k half 1
            def load_half(x_sb, src_d, h):
                # DRAM [512, 1024] col-slice -> SBUF [128, 4, 1024]
                nc.sync.dma_start(
                    x_sb[:],
                    src_d[:, h * (S // 2):(h + 1) * (S // 2)].rearrange(
                        "(c p) i -> p c i", p=128))

            nc.sync.dma_start(cst_sb[:], cst_d[:])
            load_half(xq[0], qt_d, 0)
            load_half(xk[0], kt_d, 0)
            load_half(xq[1], qt_d, 1)
            load_half(xk[1], kt_d, 1)

            # ---- PE warm-up: junk matmuls while the first DMAs fly --------
            # The HAM clock gate keeps PE at 1.2 GHz until ~3.4us of
            # sustained activity; open it before the real work lands.
            nc.vector.memset(wu[:], 0.0)
            wu_ps = ps.tile([128, 512], F32, tag="sc", name="wu_ps", bufs=4)
            for i in range(14):
                nc.tensor.matmul(wu_ps[:], lhsT=wu[:, 0:128], rhs=wu[:],
                                 start=(i == 0), stop=(i == 13))
            nc.vector.tensor_copy(wu2[:], wu_ps[:, 0:1])

            # ones columns for v' (Pool, early, no deps)
            for jb in range(NB):
                nc.gpsimd.memset(vprime[jb][:, DK:DK + 1], 1.0)

            # ---- projections ----------------------------------------------
            def query_proj(p):
                sl = slice(p * 512, (p + 1) * 512)
                h, off = divmod(p * 512, S // 2)
                pq = ps.tile([DK, 512], F32, tag="sc", name=f"pq_{p}", bufs=4)
                for cc in range(4):
                    nc.tensor.matmul(pq[:],
                                     lhsT=wq_sb[:, cc * DK:(cc + 1) * DK],
                                     rhs=xq[h][:, cc, off:off + 512],
                                     start=(cc == 0), stop=(cc == 3))
                nc.vector.tensor_scalar_add(qT[:, sl], pq[:], bq_sb[:, 0:1])

            def kv_proj(p):
                sl = slice(p * 512, (p + 1) * 512)
                h, off = divmod(p * 512, S // 2)
                pkv = ps.tile([128, 512], F32, tag="sc", name=f"pkv_{p}", bufs=4)
                for cc in range(4):
                    nc.tensor.matmul(pkv[:],
                                     lhsT=wkv_sb[:, cc * 128:(cc + 1) * 128],
                                     rhs=xk[h][:, cc, off:off + 512],
                                     start=(cc == 0), stop=(cc == 3))
                nc.vector.tensor_scalar_add(kvT[:, sl], pkv[:], bkv_sb[:, 0:1])
                # v' for the 4 j-blocks of this piece
                for t in range(4):
                    jb = p * 4 + t
                    pv = ps.tile([128, DK], BF, tag="pl", name=f"pv_{jb}", bufs=1)
                    nc.tensor.transpose(pv[:],
                                        kvT[64:128, jb * 128:(jb + 1) * 128],
                                        id_sb[64:128, 64:128])
                    nc.vector.tensor_copy(vprime[jb][:, 0:DK], pv[:])

            # ---- scores + exp for sweep p ---------------------------------
            def scores(p):
                Jmax = 4 * p + 3 if masked else NB - 1
                for J in range(0, Jmax + 1):
                    pt = pts[J]
                    i_start = max(512 * p, 128 * J) if masked else 512 * p
                    w = 512 * p + 512 - i_start
                    x0 = i_start - (128 * J if masked else 0)
                    psc = ps.tile([128, 512], F32, tag="sc", name=f"sc_{J}_{p}",
                                  bufs=4)
                    diag = masked and J // 4 == p
                    nc.tensor.matmul(psc[:, 0:w],
                                     lhsT=kvT[0:DK, J * 128:(J + 1) * 128],
                                     rhs=qT[:, i_start:i_start + w],
                                     start=True, stop=not diag,
                                     skip_group_check=True)
                    if diag:
                        # accumulate -1e30 upper-triangle into the diag block
                        nc.tensor.matmul(psc[:, 0:128], lhsT=id_sb[:],
                                         rhs=trineg_sb[:], start=False, stop=True,
                                         skip_group_check=True)
                    nc.scalar.activation(pt[:, x0:x0 + w], psc[:, 0:w],
                                         Exp, scale=0.125)

            # ---- head accumulation + finalize for sweep p -----------------
            def heads(p):
                Jmax = 4 * p + 3 if masked else NB - 1
                hacc = ps.tile([DK + 1, 512], F32, tag="ha", name=f"ha_{p}", bufs=1)
                for J in range(0, Jmax + 1):
                    b_lo = max(4 * p, J) if masked else 4 * p
                    wdt = (4 * p + 4 - b_lo) * 128
                    c0 = (b_lo % 4) * 128
                    x = (128 * (b_lo - J) if masked else 512 * p)
                    nc.tensor.matmul(hacc[:, c0:c0 + wdt],
                                     lhsT=vprime[J][:], rhs=pts[J][:, x:x + wdt],
                                     start=(J == 0), stop=(J == Jmax),
                                     skip_group_check=True)
                ht4 = hts.tile([DK + 1, 512], BF, tag="ht", name=f"ht4_{p}")
                nc.vector.tensor_copy(ht4[:], hacc[:])
                # finalize: per 128-row block, normalize + out projection
                ot4 = osb.tile([128, 4, D], BF, tag="ot", name=f"ot4_{p}")
                pl4 = ps.tile([128, 4, 2], BF, tag="pl", name=f"pl4_{p}", bufs=1)
                for t in range(4):
                    nc.tensor.transpose(pl4[:, t, 0:1],
                                        ht4[DK:DK + 1, t * 128:(t + 1) * 128],
                                        id_sb[64:65, 64:65])
                r4 = hts.tile([128, 4], F32, tag="r", name=f"r4_{p}")
                nc.vector.reciprocal(r4[:], pl4[:, :, 0])
                for b in range(4 * p, 4 * p + 4):
                    c0 = (b % 4) * 128
                    po = ps.tile([128, 512], F32, tag="po", name=f"po_{b}", bufs=2)
                    nc.tensor.matmul(po[:], lhsT=ht4[:, c0:c0 + 128], rhs=frhs_sb[:],
                                     start=True, stop=True)
                    if b % 2 == 0:
                        nc.vector.tensor_scalar_mul(ot4[:, b % 4, :], po[:],
                                                    r4[:, b % 4:b % 4 + 1])
                    else:
                        nc.scalar.mul(ot4[:, b % 4, :], po[:],
                                      r4[:, b % 4:b % 4 + 1])
                nc.sync.dma_start(
                    out_d[p * 512:(p + 1) * 512, :].rearrange(
                        "(c p) i -> p c i", p=128),
                    ot4[:])

            # ---- program: fill PE while loads land, lag heads one sweep ---
            query_proj(0)
            query_proj(1)
            kv_proj(0)
            scores(0)
            kv_proj(1)
            scores(1)
            heads(0)
            query_proj(2)
            query_proj(3)
            heads(1)
            kv_proj(2)
            scores(2)
            kv_proj(3)
            scores(3)
            heads(2)
            heads(3)
            nc.gpsimd.dma_start(warm_d[:], wu2[:])

    _split_sync_waits(nc)
    return nc


_NC_CACHE = {}


def _get_nc(masked: bool):
    if masked not in _NC_CACHE:
        _NC_CACHE[masked] = _build_nc(masked)
    return _NC_CACHE[masked]


def _prep_consts(Wq, bq, Wk, bk, Wv, bv, Wo, bo):
    Wq = np.asarray(Wq, dtype=np.float64)
    Wk = np.asarray(Wk, dtype=np.float64)
    Wv = np.asarray(Wv, dtype=np.float64)
    Wo = np.asarray(Wo, dtype=np.float64)
    bq_h = np.asarray(bq, dtype=np.float32).reshape(DK, 1)
    bk_h = np.asarray(bk, dtype=np.float32).reshape(DK, 1)
    bv_h = np.asarray(bv, dtype=np.float32).reshape(DK, 1)
    bo_h = np.asarray(bo, dtype=np.float64)
    wo_eff = Wo.reshape(H, DK, D).sum(axis=0)
    frhs_h = np.concatenate([wo_eff, bo_h[None, :]], axis=0).astype(ml_dtypes.bfloat16)
    jj, ii = np.meshgrid(np.arange(128), np.arange(128), indexing="ij")
    trineg_h = np.where(jj <= ii, 0.0, -1e30).astype(ml_dtypes.bfloat16)
    # pack everything into one [128, 1540] bf16 tensor (single DMA)
    cst = np.zeros((128, 1540), dtype=ml_dtypes.bfloat16)
    wq_bf = Wq.astype(ml_dtypes.bfloat16)          # [512, 64]
    wkv_bf = np.concatenate([Wk, Wv], axis=1).astype(ml_dtypes.bfloat16)  # [512,128]
    for cc in range(4):
        cst[:, cc * DK:(cc + 1) * DK] = wq_bf[cc * 128:(cc + 1) * 128]
        cst[:, 256 + cc * 128:256 + (cc + 1) * 128] = wkv_bf[cc * 128:(cc + 1) * 128]
    cst[0:DK + 1, 768:1280] = frhs_h
    cst[:, 1280:1408] = trineg_h
    cst[:, 1408:1536] = np.eye(128, dtype=ml_dtypes.bfloat16)
    cst[0:DK, 1536:1538] = np.ascontiguousarray(bq_h).view(ml_dtypes.bfloat16)
    bkv_f = np.ascontiguousarray(np.concatenate([bk_h, bv_h], axis=0))
    cst[:, 1538:1540] = bkv_f.view(ml_dtypes.bfloat16)
    return {"cst": cst}


# ---------------------------------------------------------------------------
def kernel(query, key, value, Wq, bq, Wk, bk, Wv, bv, Wo, bo, training):
    query = np.asarray(query, dtype=np.float32)
    key = np.asarray(key, dtype=np.float32)
    masked = bool(np.asarray(training).item())
    B = query.shape[0]

    consts = _prep_consts(Wq, bq, Wk, bk, Wv, bv, Wo, bo)
    in_maps = [
        dict(consts,
             qt=np.ascontiguousarray(query[i].T).astype(ml_dtypes.bfloat16),
             kt=np.ascontiguousarray(key[i].T).astype(ml_dtypes.bfloat16))
        for i in range(B)
    ]

    nc = _get_nc(masked)
    res = run_bass_kernel_spmd(nc, in_maps, core_ids=list(range(B)))
    return np.stack([np.asarray(res.results[i]["out"]).astype(np.float32)
                     for i in range(B)])
